# revision 15
# baseline (speedup 1.0000x reference)
"""3-layer GCN (PyG GCNConv-style) on 8 Trainium2 NeuronCores.

Distribution: 1-D node partition (2048 nodes per core). Per core:
  - GEMM1: h1T[36,2048] = W1^T @ x[rows_c]^T with x and W1 (prescaled x128)
    in fp8e4m3, PE DoubleRow perf mode (K=256 per pass, 0.5 cyc/row).
  - Per layer: messages g = s_pre * dis * h are quantized to fp8, AllGathered,
    and aggregated against a dense per-core adjacency block A[16384, 2048]
    in fp8 (entries = exact edge multiplicities incl. self-loops; the
    symmetric norm is folded into dis pre/post scaling), via DoubleRow PE
    matmuls accumulating aggT[36, 2048] in PSUM. The first NCACHE 256-row
    chunks of A stay resident in SBUF after layer 1; only the tail is
    re-streamed for layers 2 and 3.
  - Small GEMMs (W2, W3) in fp32, softmax over the 16 classes at the end.
"""
import numpy as np
import concourse.bacc as bacc
import concourse.mybir as mybir
import concourse.tile as tile
from concourse.bass_utils import run_bass_kernel_spmd

N = 16384
E = 524288
H = 36
C = 16
NCORES = 8
ND = N // NCORES          # 2048 nodes per core
KC2 = N // 256            # 64 contraction chunks of 256 nodes (DoubleRow)
RB = ND // 128            # 16 row-blocks per core
HP = 48                   # H padded to mult-of-16 (dual-fp8 ldweights needs M%16==0)
NCACHE = 34               # A chunks kept SBUF-resident after layer 1
FP8 = mybir.dt.np(mybir.dt.float8e4)
DR = mybir.MatmulPerfMode.DoubleRow

# per-layer message quantization scales (power of two; see numerics_stat.py)
S_W1 = 128.0                       # W1 prescale so fp8 weights are normal-range
S_PRE = (1.0 / 8.0, 64.0, 128.0)   # applied to gT (which carries S_W1/prior scale)
# post-agg de-scale: layer0 carries S_W1 * S_PRE[0] = 16, others carry S_PRE[l]
S_POST = (1.0 / 16.0, 1.0 / 64.0, 1.0 / 128.0)

_PROGRAM = None
_LAST_RES = None


def _build_program():
    nc = bacc.Bacc(None)
    f32, fp8 = mybir.dt.float32, mybir.dt.float8e4

    xT_d = nc.dram_tensor("xT", [KC2, 128, 2, ND], fp8, kind="ExternalInput")
    W1c_d = nc.dram_tensor("W1c", [128, KC2 * 2 * HP], fp8, kind="ExternalInput")
    A_d = nc.dram_tensor("A", [KC2, 128, 2, ND], fp8, kind="ExternalInput")
    disr_d = nc.dram_tensor("disr", [H, ND], f32, kind="ExternalInput")
    W2_d = nc.dram_tensor("W2", [H, H], f32, kind="ExternalInput")
    W3_d = nc.dram_tensor("W3", [H, C], f32, kind="ExternalInput")
    b1_d = nc.dram_tensor("b1", [H, 1], f32, kind="ExternalInput")
    b2_d = nc.dram_tensor("b2", [H, 1], f32, kind="ExternalInput")
    b3_d = nc.dram_tensor("b3", [C, 1], f32, kind="ExternalInput")
    I_d = nc.dram_tensor("ident", [128, 128], f32, kind="ExternalInput")
    out_d = nc.dram_tensor("out", [ND, C], f32, kind="ExternalOutput")

    cc_in = [nc.dram_tensor(f"cc{l}_in", [128, RB * HP], fp8)
             for l in range(3)]
    cc_out = [nc.dram_tensor(f"cc{l}_out", [NCORES * 128, RB * HP], fp8,
                             addr_space="Shared")
              for l in range(3)]
    groups = [list(range(NCORES))]

    with tile.TileContext(nc) as tc:
        with (
            tc.tile_pool(name="const", bufs=1) as constp,
            tc.tile_pool(name="acache", bufs=1) as acp,
            tc.tile_pool(name="mv", bufs=3) as mvp,
            tc.tile_pool(name="gt", bufs=1) as gtp,
            tc.tile_pool(name="work", bufs=1) as work,
            tc.tile_pool(name="psb", bufs=1, space="PSUM") as psb,
            tc.tile_pool(name="pst", bufs=2, space="PSUM") as pst,
        ):
            W1c = constp.tile([128, KC2, 2, HP], fp8)
            disr = constp.tile([H, ND], f32)
            W2t = constp.tile([H, H], f32)
            W3t = constp.tile([H, C], f32)
            b1t = constp.tile([H, 1], f32)
            b2t = constp.tile([H, 1], f32)
            b3t = constp.tile([C, 1], f32)
            ident = constp.tile([128, 128], f32)
            nc.sync.dma_start(W1c[:], W1c_d[:].rearrange("p (c t f) -> p c t f",
                                                         t=2, f=HP))
            nc.sync.dma_start(disr[:], disr_d[:])
            nc.sync.dma_start(W2t[:], W2_d[:])
            nc.sync.dma_start(W3t[:], W3_d[:])
            nc.sync.dma_start(b1t[:], b1_d[:])
            nc.sync.dma_start(b2t[:], b2_d[:])
            nc.sync.dma_start(b3t[:], b3_d[:])
            nc.sync.dma_start(ident[:], I_d[:])

            acache = acp.tile([128, NCACHE, 2, ND], fp8)
            g_t = gtp.tile([128, KC2, 2, HP], fp8, tag="g")
            gown = work.tile([128, RB, HP], fp8, tag="gown")
            nc.vector.memset(gown[:, :, H:HP], 0.0)

            # ---- GEMM1: h1T[36, 2048] += W1[cc]^T @ xT[cc]  (DoubleRow) ----
            hT = psb.tile([HP, ND], f32, tag="big")
            for cp in range(KC2 // 2):
                xt = mvp.tile([128, 2, 2, ND], fp8, tag="mv")
                nc.sync.dma_start(
                    xt[:],
                    xT_d[2 * cp:2 * cp + 2, :, :, :].rearrange(
                        "c p t d -> p c t d"),
                )
                for i in range(2):
                    cc = 2 * cp + i
                    for q in range(4):
                        nc.tensor.matmul(
                            hT[0:HP, q * 512:(q + 1) * 512],
                            W1c[:, cc, :, :],
                            xt[:, i, :, q * 512:(q + 1) * 512],
                            start=(cc == 0),
                            stop=(cc == KC2 - 1),
                            perf_mode=DR,
                        )

            # prefetch the A cache fills now: they flow on SP during the
            # layer-0 gather window while PE is idle on the collective
            for cf in range(NCACHE // 2):
                nc.sync.dma_start(
                    acache[:, 2 * cf:2 * cf + 2, :, :],
                    A_d[2 * cf:2 * cf + 2, :, :, :].rearrange(
                        "c p t d -> p c t d"),
                )

            for layer in range(3):
                F = H if layer < 2 else C
                # ---- prescale by dis (own rows), quantize, share ----
                gT = work.tile([H, ND], f32, tag="w1")
                nc.vector.tensor_tensor(
                    gT[0:F, :], hT[0:F, :], disr[0:F, :], mybir.AluOpType.mult
                )
                if layer == 2:
                    nc.vector.memset(gown[:, :, C:H], 0.0)
                for rb in range(RB):
                    sl = slice(rb * 128, (rb + 1) * 128)
                    tp = pst.tile([128, H], f32, tag="tp")
                    nc.tensor.transpose(
                        tp[:, 0:F],
                        gT[0:F, sl],
                        ident[0:F, 0:F],
                    )
                    nc.vector.tensor_scalar(
                        gown[:, rb, 0:F], tp[:, 0:F],
                        float(S_PRE[layer]), None, mybir.AluOpType.mult,
                    )
                nc.scalar.dma_start(
                    cc_in[layer][:].rearrange("p (r f) -> p r f", f=HP), gown[:]
                )
                nc.gpsimd.collective_compute(
                    "AllGather",
                    mybir.AluOpType.bypass,
                    replica_groups=groups,
                    ins=[cc_in[layer][:]],
                    outs=[cc_out[layer][:]],
                )
                nc.scalar.dma_start(
                    g_t[:].rearrange("p (c lt) t f -> p c (lt t f)", c=8),
                    cc_out[layer][:].rearrange("(c p) ltf -> p c ltf", p=128),
                )

                # ---- dense aggregation: aggT[F, 2048] += g[cc]^T @ A[cc] ----
                # streamed chunks first (fresh DMA), cached chunks last: frees
                # stream buffers early so the next layer's prefetch can run
                # during this layer's cached phase and the next collective.
                aggT = psb.tile([HP, ND], f32, tag="big")
                FS = HP
                nmm = 0

                def agg_mm(cc, a_ap):
                    nonlocal nmm
                    for q in range(4):
                        nc.tensor.matmul(
                            aggT[0:FS, q * 512:(q + 1) * 512],
                            g_t[:, cc, :, 0:FS],
                            a_ap[:, :, q * 512:(q + 1) * 512],
                            start=(nmm == 0),
                            stop=(nmm == KC2 - 1),
                            perf_mode=DR,
                        )
                    nmm += 1

                def stream_pair(sp):
                    at = mvp.tile([128, 2, 2, ND], fp8, tag="mv")
                    c0 = NCACHE + 2 * sp
                    nc.sync.dma_start(
                        at[:],
                        A_d[c0:c0 + 2, :, :, :].rearrange("c p t d -> p c t d"),
                    )
                    for i in range(2):
                        agg_mm(c0 + i, at[:, i, :, :])

                # interleave ~2 cached chunks per streamed pair so the PE
                # never outruns the stream DMA; prime with cached chunks
                npairs = (KC2 - NCACHE) // 2
                k = 0
                for sp in range(npairs):
                    for _ in range(2):
                        if k < NCACHE:
                            agg_mm(k, acache[:, k, :, :])
                            k += 1
                    stream_pair(sp)
                while k < NCACHE:
                    agg_mm(k, acache[:, k, :, :])
                    k += 1

                if layer < 2:
                    # in_{l+1}T = relu(s_post * (dis*aggT) + b)
                    tmp = work.tile([H, ND], f32, tag="w2")
                    nc.vector.tensor_tensor(
                        tmp[:], aggT[0:H, :], disr[:], mybir.AluOpType.mult
                    )
                    inT = work.tile([H, ND], f32, tag="w1")
                    nc.scalar.activation(
                        inT[:], tmp[:], mybir.ActivationFunctionType.Relu,
                        bias=b1t[:] if layer == 0 else b2t[:],
                        scale=float(S_POST[layer]),
                    )
                    # next-layer GEMM: hT = W^T @ inT  (K = 36, fp32)
                    Wt = W2t if layer == 0 else W3t
                    Fn = H if layer == 0 else C
                    hT = psb.tile([HP, ND], f32, tag="big")
                    for q in range(4):
                        nc.tensor.matmul(
                            hT[0:Fn, q * 512:(q + 1) * 512],
                            Wt[:],
                            inT[:, q * 512:(q + 1) * 512],
                            start=True,
                            stop=True,
                        )
                else:
                    # logitsT = s_post * (dis*aggT) + b3 ; softmax over classes
                    tmp = work.tile([H, ND], f32, tag="w2")
                    nc.vector.tensor_tensor(
                        tmp[0:C, :], aggT[0:C, :], disr[0:C, :],
                        mybir.AluOpType.mult,
                    )
                    logT = work.tile([C, ND], f32, tag="w1")
                    nc.vector.tensor_scalar(
                        logT[:], tmp[0:C, :],
                        float(S_POST[layer]), b3t[:],
                        mybir.AluOpType.mult, mybir.AluOpType.add,
                    )
                    # transpose to natural [2048, 16]
                    onat = work.tile([128, RB, C], f32, tag="onat")
                    for rb in range(RB):
                        tp = pst.tile([128, H], f32, tag="tp")
                        nc.tensor.transpose(
                            tp[:, 0:C],
                            logT[:, rb * 128:(rb + 1) * 128],
                            ident[0:C, 0:C],
                        )
                        nc.vector.tensor_copy(onat[:, rb, :], tp[:, 0:C])
                    # softmax along the class (free) dim
                    negmax = work.tile([128, RB], f32, tag="negmax")
                    nc.vector.tensor_reduce(
                        negmax[:], onat[:], axis=mybir.AxisListType.X,
                        op=mybir.AluOpType.max, negate=True,
                    )
                    expv = work.tile([128, RB, C], f32, tag="expv")
                    ssum = work.tile([128, RB], f32, tag="ssum")
                    for rb in range(RB):
                        nc.scalar.activation(
                            expv[:, rb, :], onat[:, rb, :],
                            mybir.ActivationFunctionType.Exp,
                            bias=negmax[:, rb:rb + 1],
                            accum_out=ssum[:, rb:rb + 1],
                        )
                    rsum = work.tile([128, RB], f32, tag="rsum")
                    nc.vector.reciprocal(rsum[:], ssum[:])
                    prob = work.tile([128, RB, C], f32, tag="onat")
                    for rb in range(RB):
                        nc.vector.tensor_scalar(
                            prob[:, rb, :], expv[:, rb, :],
                            rsum[:, rb:rb + 1], None, mybir.AluOpType.mult,
                        )
                    nc.scalar.dma_start(
                        out_d[:].rearrange("(b p) f -> p b f", p=128), prob[:]
                    )

    nc.finalize()
    return nc


def _get_program():
    global _PROGRAM
    if _PROGRAM is None:
        _PROGRAM = _build_program()
    return _PROGRAM


def kernel(x, edge_index, W1, b1, W2, b2, W3, b3, _profile=False):
    x = np.asarray(x, dtype=np.float32)
    edge_index = np.asarray(edge_index)
    W1 = np.asarray(W1, dtype=np.float32)
    W2 = np.asarray(W2, dtype=np.float32)
    W3 = np.asarray(W3, dtype=np.float32)
    b1 = np.asarray(b1, dtype=np.float32)
    b2 = np.asarray(b2, dtype=np.float32)
    b3 = np.asarray(b3, dtype=np.float32)

    # ---- graph preprocessing (host) ----
    loop = np.arange(N, dtype=np.int64)
    src = np.concatenate([edge_index[0].astype(np.int64), loop])
    dst = np.concatenate([edge_index[1].astype(np.int64), loop])
    deg = np.bincount(dst, minlength=N).astype(np.float32)
    dis = (1.0 / np.sqrt(np.maximum(deg, np.float32(1.0)))).astype(np.float32)

    # W1 chunked for DoubleRow: [p, cc, t, f] with src row = 256*cc + 128*t + p
    W1p = np.zeros((N, HP), np.float32)
    W1p[:, 0:H] = W1 * np.float32(S_W1)
    W1c = np.ascontiguousarray(
        W1p.astype(FP8)
        .reshape(KC2, 2, 128, HP).transpose(2, 0, 1, 3)
    ).reshape(128, KC2 * 2 * HP)
    ident = np.eye(128, dtype=np.float32)

    core_dst = dst // ND
    in_maps = []
    for c in range(NCORES):
        m = core_dst == c
        # A[src, local_dst] edge multiplicities, chunked [cc, p, t, d]
        lin = src[m] * ND + (dst[m] - c * ND)
        cnt = np.bincount(lin, minlength=N * ND).astype(np.float32)
        A8 = np.ascontiguousarray(
            cnt.astype(FP8).reshape(KC2, 2, 128, ND).transpose(0, 2, 1, 3)
        )
        xT = np.ascontiguousarray(
            x[c * ND:(c + 1) * ND, :].astype(FP8).T
            .reshape(KC2, 2, 128, ND).transpose(0, 2, 1, 3)
        )
        disr = np.ascontiguousarray(
            np.broadcast_to(dis[c * ND:(c + 1) * ND][None, :], (H, ND))
        )
        in_maps.append({
            "xT": xT,
            "W1c": W1c,
            "A": A8,
            "disr": disr,
            "W2": W2,
            "W3": W3,
            "b1": b1.reshape(H, 1),
            "b2": b2.reshape(H, 1),
            "b3": b3.reshape(C, 1),
            "ident": ident,
        })

    nc = _get_program()
    global _LAST_RES
    res = run_bass_kernel_spmd(nc, in_maps, list(range(NCORES)),
                               trace=bool(_profile))
    _LAST_RES = res
    out = np.concatenate([res.results[c]["out"] for c in range(NCORES)], axis=0)
    if _profile:
        return out, res.exec_time_ns
    return out


# revision 17
# speedup vs baseline: 1.0103x; 1.0103x over previous
"""3-layer GCN (PyG GCNConv-style) on 8 Trainium2 NeuronCores.

Distribution: 1-D node partition (2048 nodes per core). Per core:
  - GEMM1: h1T[36,2048] = W1^T @ x[rows_c]^T with x and W1 (prescaled x128)
    in fp8e4m3, PE DoubleRow perf mode (K=256 per pass, 0.5 cyc/row).
  - Per layer: messages g = s_pre * dis * h are quantized to fp8, AllGathered,
    and aggregated against a dense per-core adjacency block A[16384, 2048]
    in fp8 (entries = exact edge multiplicities incl. self-loops; the
    symmetric norm is folded into dis pre/post scaling), via DoubleRow PE
    matmuls accumulating aggT[36, 2048] in PSUM. The first NCACHE 256-row
    chunks of A stay resident in SBUF after layer 1; only the tail is
    re-streamed for layers 2 and 3.
  - Small GEMMs (W2, W3) in fp32, softmax over the 16 classes at the end.
"""
import numpy as np
import concourse.bacc as bacc
import concourse.mybir as mybir
import concourse.tile as tile
from concourse.bass_utils import run_bass_kernel_spmd

N = 16384
E = 524288
H = 36
C = 16
NCORES = 8
ND = N // NCORES          # 2048 nodes per core
KC2 = N // 256            # 64 contraction chunks of 256 nodes (DoubleRow)
RB = ND // 128            # 16 row-blocks per core
HP = 48                   # H padded to mult-of-16 (dual-fp8 ldweights needs M%16==0)
NCACHE = 34               # A chunks kept SBUF-resident after layer 1
FP8 = mybir.dt.np(mybir.dt.float8e4)
DR = mybir.MatmulPerfMode.DoubleRow

# per-layer message quantization scales (power of two; see numerics_stat.py)
S_W1 = 128.0                       # W1 prescale so fp8 weights are normal-range
S_PRE = (1.0 / 8.0, 64.0, 128.0)   # applied to gT (which carries S_W1/prior scale)
# post-agg de-scale: layer0 carries S_W1 * S_PRE[0] = 16, others carry S_PRE[l]
S_POST = (1.0 / 16.0, 1.0 / 64.0, 1.0 / 128.0)

_PROGRAM = None
_LAST_RES = None


def _build_program():
    nc = bacc.Bacc(None)
    f32, fp8 = mybir.dt.float32, mybir.dt.float8e4

    xT_d = nc.dram_tensor("xT", [KC2, 128, 2, ND], fp8, kind="ExternalInput")
    W1c_d = nc.dram_tensor("W1c", [128, KC2 * 2 * HP], fp8, kind="ExternalInput")
    A_d = nc.dram_tensor("A", [KC2, 128, 2, ND], fp8, kind="ExternalInput")
    disr_d = nc.dram_tensor("disr", [H, ND], f32, kind="ExternalInput")
    W2_d = nc.dram_tensor("W2", [H, H], f32, kind="ExternalInput")
    W3_d = nc.dram_tensor("W3", [H, C], f32, kind="ExternalInput")
    b1_d = nc.dram_tensor("b1", [H, 1], f32, kind="ExternalInput")
    b2_d = nc.dram_tensor("b2", [H, 1], f32, kind="ExternalInput")
    b3_d = nc.dram_tensor("b3", [C, 1], f32, kind="ExternalInput")
    I_d = nc.dram_tensor("ident", [128, 128], f32, kind="ExternalInput")
    out_d = nc.dram_tensor("out", [ND, C], f32, kind="ExternalOutput")

    HRB = RB // 2
    cc_in = [[nc.dram_tensor(f"cc{l}{h}_in", [128, HRB * HP], fp8)
              for h in range(2)] for l in range(3)]
    cc_out = [[nc.dram_tensor(f"cc{l}{h}_out", [NCORES * 128, HRB * HP], fp8,
                              addr_space="Shared") for h in range(2)]
              for l in range(3)]
    groups = [list(range(NCORES))]

    with tile.TileContext(nc) as tc:
        with (
            tc.tile_pool(name="const", bufs=1) as constp,
            tc.tile_pool(name="acache", bufs=1) as acp,
            tc.tile_pool(name="mv", bufs=3) as mvp,
            tc.tile_pool(name="gt", bufs=1) as gtp,
            tc.tile_pool(name="work", bufs=1) as work,
            tc.tile_pool(name="psb", bufs=1, space="PSUM") as psb,
            tc.tile_pool(name="pst", bufs=2, space="PSUM") as pst,
        ):
            W1c = constp.tile([128, KC2, 2, HP], fp8)
            disr = constp.tile([H, ND], f32)
            W2t = constp.tile([H, H], f32)
            W3t = constp.tile([H, C], f32)
            b1t = constp.tile([H, 1], f32)
            b2t = constp.tile([H, 1], f32)
            b3t = constp.tile([C, 1], f32)
            ident = constp.tile([128, 128], f32)
            nc.sync.dma_start(W1c[:], W1c_d[:].rearrange("p (c t f) -> p c t f",
                                                         t=2, f=HP))
            nc.sync.dma_start(disr[:], disr_d[:])
            nc.sync.dma_start(W2t[:], W2_d[:])
            nc.sync.dma_start(W3t[:], W3_d[:])
            nc.sync.dma_start(b1t[:], b1_d[:])
            nc.sync.dma_start(b2t[:], b2_d[:])
            nc.sync.dma_start(b3t[:], b3_d[:])
            nc.sync.dma_start(ident[:], I_d[:])

            acache = acp.tile([128, NCACHE, 2, ND], fp8)
            # g table split by own-node half: chunk cc=(c,l) is in half l//4
            g_th = [gtp.tile([128, 8, 4, 2, HP], fp8, tag=f"g{h}",
                                 name=f"g_th{h}")
                    for h in range(2)]

            def g_lhsT(cc):
                c, l = cc // 8, cc % 8
                return g_th[l // 4][:, c, l % 4, :, :]

            gown = work.tile([128, RB, HP], fp8, tag="gown")
            nc.vector.memset(gown[:, :, H:HP], 0.0)

            # ---- GEMM1: h1T[36, 2048] += W1[cc]^T @ xT[cc]  (DoubleRow) ----
            hT = psb.tile([HP, ND], f32, tag="big")
            for cp in range(KC2 // 2):
                xt = mvp.tile([128, 2, 2, ND], fp8, tag="mv")
                nc.sync.dma_start(
                    xt[:],
                    xT_d[2 * cp:2 * cp + 2, :, :, :].rearrange(
                        "c p t d -> p c t d"),
                )
                for i in range(2):
                    cc = 2 * cp + i
                    for q in range(4):
                        nc.tensor.matmul(
                            hT[0:HP, q * 512:(q + 1) * 512],
                            W1c[:, cc, :, :],
                            xt[:, i, :, q * 512:(q + 1) * 512],
                            start=(cc == 0),
                            stop=(cc == KC2 - 1),
                            perf_mode=DR,
                        )

            # prefetch the A cache fills now: they flow on SP during the
            # layer-0 gather window while PE is idle on the collective
            for cf in range(NCACHE // 2):
                nc.sync.dma_start(
                    acache[:, 2 * cf:2 * cf + 2, :, :],
                    A_d[2 * cf:2 * cf + 2, :, :, :].rearrange(
                        "c p t d -> p c t d"),
                )

            for layer in range(3):
                F = H if layer < 2 else C
                # ---- prescale by dis (own rows), quantize, share ----
                gT = work.tile([H, ND], f32, tag="w1")
                nc.vector.tensor_tensor(
                    gT[0:F, :], hT[0:F, :], disr[0:F, :], mybir.AluOpType.mult
                )
                if layer == 2:
                    nc.vector.memset(gown[:, :, C:H], 0.0)
                for h in range(2):
                    for rb in range(h * HRB, (h + 1) * HRB):
                        sl = slice(rb * 128, (rb + 1) * 128)
                        tp = pst.tile([128, H], f32, tag="tp")
                        nc.tensor.transpose(
                            tp[:, 0:F],
                            gT[0:F, sl],
                            ident[0:F, 0:F],
                        )
                        nc.vector.tensor_scalar(
                            gown[:, rb, 0:F], tp[:, 0:F],
                            float(S_PRE[layer]), None, mybir.AluOpType.mult,
                        )
                    nc.scalar.dma_start(
                        cc_in[layer][h][:].rearrange("p (r f) -> p r f", f=HP),
                        gown[:, h * HRB:(h + 1) * HRB, :],
                    )
                    nc.gpsimd.collective_compute(
                        "AllGather",
                        mybir.AluOpType.bypass,
                        replica_groups=groups,
                        ins=[cc_in[layer][h][:]],
                        outs=[cc_out[layer][h][:]],
                    )
                for h in range(2):
                    nc.scalar.dma_start(
                        g_th[h][:].rearrange("p c l t f -> p c (l t f)"),
                        cc_out[layer][h][:].rearrange("(c p) ltf -> p c ltf",
                                                      p=128),
                    )


                # ---- dense aggregation: aggT[F, 2048] += g[cc]^T @ A[cc] ----
                # streamed chunks first (fresh DMA), cached chunks last: frees
                # stream buffers early so the next layer's prefetch can run
                # during this layer's cached phase and the next collective.
                aggT = psb.tile([HP, ND], f32, tag="big")
                FS = HP
                nmm = 0

                def agg_mm(cc, a_ap):
                    nonlocal nmm
                    for q in range(4):
                        nc.tensor.matmul(
                            aggT[0:FS, q * 512:(q + 1) * 512],
                            g_lhsT(cc),
                            a_ap[:, :, q * 512:(q + 1) * 512],
                            start=(nmm == 0),
                            stop=(nmm == KC2 - 1),
                            perf_mode=DR,
                        )
                    nmm += 1

                def stream_pair(sp):
                    at = mvp.tile([128, 2, 2, ND], fp8, tag="mv")
                    c0 = NCACHE + 2 * sp
                    nc.sync.dma_start(
                        at[:],
                        A_d[c0:c0 + 2, :, :, :].rearrange("c p t d -> p c t d"),
                    )
                    for i in range(2):
                        agg_mm(c0 + i, at[:, i, :, :])

                # process half-0 chunks (need only collective A) then half-1;
                # within each: interleave ~2 cached chunks per streamed pair
                npairs = (KC2 - NCACHE) // 2
                for h in range(2):
                    cached = [cc for cc in range(NCACHE) if (cc % 8) // 4 == h]
                    pairs = [sp for sp in range(npairs)
                             if ((NCACHE + 2 * sp) % 8) // 4 == h]
                    k = 0
                    for sp in pairs:
                        for _ in range(2):
                            if k < len(cached):
                                agg_mm(cached[k], acache[:, cached[k], :, :])
                                k += 1
                        stream_pair(sp)
                    while k < len(cached):
                        agg_mm(cached[k], acache[:, cached[k], :, :])
                        k += 1

                if layer < 2:
                    # in_{l+1}T = relu(s_post * (dis*aggT) + b)
                    tmp = work.tile([H, ND], f32, tag="w2")
                    nc.vector.tensor_tensor(
                        tmp[:], aggT[0:H, :], disr[:], mybir.AluOpType.mult
                    )
                    inT = work.tile([H, ND], f32, tag="w1")
                    nc.scalar.activation(
                        inT[:], tmp[:], mybir.ActivationFunctionType.Relu,
                        bias=b1t[:] if layer == 0 else b2t[:],
                        scale=float(S_POST[layer]),
                    )
                    # next-layer GEMM: hT = W^T @ inT  (K = 36, fp32)
                    Wt = W2t if layer == 0 else W3t
                    Fn = H if layer == 0 else C
                    hT = psb.tile([HP, ND], f32, tag="big")
                    for q in range(4):
                        nc.tensor.matmul(
                            hT[0:Fn, q * 512:(q + 1) * 512],
                            Wt[:],
                            inT[:, q * 512:(q + 1) * 512],
                            start=True,
                            stop=True,
                        )
                else:
                    # logitsT = s_post * (dis*aggT) + b3 ; softmax over classes
                    tmp = work.tile([H, ND], f32, tag="w2")
                    nc.vector.tensor_tensor(
                        tmp[0:C, :], aggT[0:C, :], disr[0:C, :],
                        mybir.AluOpType.mult,
                    )
                    logT = work.tile([C, ND], f32, tag="w1")
                    nc.vector.tensor_scalar(
                        logT[:], tmp[0:C, :],
                        float(S_POST[layer]), b3t[:],
                        mybir.AluOpType.mult, mybir.AluOpType.add,
                    )
                    # transpose to natural [2048, 16]
                    onat = work.tile([128, RB, C], f32, tag="onat")
                    for rb in range(RB):
                        tp = pst.tile([128, H], f32, tag="tp")
                        nc.tensor.transpose(
                            tp[:, 0:C],
                            logT[:, rb * 128:(rb + 1) * 128],
                            ident[0:C, 0:C],
                        )
                        nc.vector.tensor_copy(onat[:, rb, :], tp[:, 0:C])
                    # softmax along the class (free) dim
                    negmax = work.tile([128, RB], f32, tag="negmax")
                    nc.vector.tensor_reduce(
                        negmax[:], onat[:], axis=mybir.AxisListType.X,
                        op=mybir.AluOpType.max, negate=True,
                    )
                    expv = work.tile([128, RB, C], f32, tag="expv")
                    ssum = work.tile([128, RB], f32, tag="ssum")
                    for rb in range(RB):
                        nc.scalar.activation(
                            expv[:, rb, :], onat[:, rb, :],
                            mybir.ActivationFunctionType.Exp,
                            bias=negmax[:, rb:rb + 1],
                            accum_out=ssum[:, rb:rb + 1],
                        )
                    rsum = work.tile([128, RB], f32, tag="rsum")
                    nc.vector.reciprocal(rsum[:], ssum[:])
                    prob = work.tile([128, RB, C], f32, tag="onat")
                    for rb in range(RB):
                        nc.vector.tensor_scalar(
                            prob[:, rb, :], expv[:, rb, :],
                            rsum[:, rb:rb + 1], None, mybir.AluOpType.mult,
                        )
                    nc.scalar.dma_start(
                        out_d[:].rearrange("(b p) f -> p b f", p=128), prob[:]
                    )

    nc.finalize()
    return nc


def _get_program():
    global _PROGRAM
    if _PROGRAM is None:
        _PROGRAM = _build_program()
    return _PROGRAM


def kernel(x, edge_index, W1, b1, W2, b2, W3, b3, _profile=False):
    x = np.asarray(x, dtype=np.float32)
    edge_index = np.asarray(edge_index)
    W1 = np.asarray(W1, dtype=np.float32)
    W2 = np.asarray(W2, dtype=np.float32)
    W3 = np.asarray(W3, dtype=np.float32)
    b1 = np.asarray(b1, dtype=np.float32)
    b2 = np.asarray(b2, dtype=np.float32)
    b3 = np.asarray(b3, dtype=np.float32)

    # ---- graph preprocessing (host) ----
    loop = np.arange(N, dtype=np.int64)
    src = np.concatenate([edge_index[0].astype(np.int64), loop])
    dst = np.concatenate([edge_index[1].astype(np.int64), loop])
    deg = np.bincount(dst, minlength=N).astype(np.float32)
    dis = (1.0 / np.sqrt(np.maximum(deg, np.float32(1.0)))).astype(np.float32)

    # W1 chunked for DoubleRow: [p, cc, t, f] with src row = 256*cc + 128*t + p
    W1p = np.zeros((N, HP), np.float32)
    W1p[:, 0:H] = W1 * np.float32(S_W1)
    W1c = np.ascontiguousarray(
        W1p.astype(FP8)
        .reshape(KC2, 2, 128, HP).transpose(2, 0, 1, 3)
    ).reshape(128, KC2 * 2 * HP)
    ident = np.eye(128, dtype=np.float32)

    core_dst = dst // ND
    in_maps = []
    for c in range(NCORES):
        m = core_dst == c
        # A[src, local_dst] edge multiplicities, chunked [cc, p, t, d]
        lin = src[m] * ND + (dst[m] - c * ND)
        cnt = np.bincount(lin, minlength=N * ND).astype(np.float32)
        A8 = np.ascontiguousarray(
            cnt.astype(FP8).reshape(KC2, 2, 128, ND).transpose(0, 2, 1, 3)
        )
        xT = np.ascontiguousarray(
            x[c * ND:(c + 1) * ND, :].astype(FP8).T
            .reshape(KC2, 2, 128, ND).transpose(0, 2, 1, 3)
        )
        disr = np.ascontiguousarray(
            np.broadcast_to(dis[c * ND:(c + 1) * ND][None, :], (H, ND))
        )
        in_maps.append({
            "xT": xT,
            "W1c": W1c,
            "A": A8,
            "disr": disr,
            "W2": W2,
            "W3": W3,
            "b1": b1.reshape(H, 1),
            "b2": b2.reshape(H, 1),
            "b3": b3.reshape(C, 1),
            "ident": ident,
        })

    nc = _get_program()
    global _LAST_RES
    res = run_bass_kernel_spmd(nc, in_maps, list(range(NCORES)),
                               trace=bool(_profile))
    _LAST_RES = res
    out = np.concatenate([res.results[c]["out"] for c in range(NCORES)], axis=0)
    if _profile:
        return out, res.exec_time_ns
    return out


# revision 18
# speedup vs baseline: 1.0154x; 1.0051x over previous
"""3-layer GCN (PyG GCNConv-style) on 8 Trainium2 NeuronCores.

Distribution: 1-D node partition (2048 nodes per core). Per core:
  - GEMM1: h1T[36,2048] = W1^T @ x[rows_c]^T with x and W1 (prescaled x128)
    in fp8e4m3, PE DoubleRow perf mode (K=256 per pass, 0.5 cyc/row).
  - Per layer: messages g = s_pre * dis * h are quantized to fp8, AllGathered,
    and aggregated against a dense per-core adjacency block A[16384, 2048]
    in fp8 (entries = exact edge multiplicities incl. self-loops; the
    symmetric norm is folded into dis pre/post scaling), via DoubleRow PE
    matmuls accumulating aggT[36, 2048] in PSUM. The first NCACHE 256-row
    chunks of A stay resident in SBUF after layer 1; only the tail is
    re-streamed for layers 2 and 3.
  - Small GEMMs (W2, W3) in fp32, softmax over the 16 classes at the end.
"""
import numpy as np
import concourse.bacc as bacc
import concourse.mybir as mybir
import concourse.tile as tile
from concourse.bass_utils import run_bass_kernel_spmd

N = 16384
E = 524288
H = 36
C = 16
NCORES = 8
ND = N // NCORES          # 2048 nodes per core
KC2 = N // 256            # 64 contraction chunks of 256 nodes (DoubleRow)
RB = ND // 128            # 16 row-blocks per core
HP = 48                   # H padded to mult-of-16 (dual-fp8 ldweights needs M%16==0)
NCACHE = 32               # A chunks kept SBUF-resident after layer 1
FP8 = mybir.dt.np(mybir.dt.float8e4)
DR = mybir.MatmulPerfMode.DoubleRow

# per-layer message quantization scales (power of two; see numerics_stat.py)
S_W1 = 128.0                       # W1 prescale so fp8 weights are normal-range
S_PRE = (1.0 / 8.0, 64.0, 128.0)   # applied to gT (which carries S_W1/prior scale)
# post-agg de-scale: layer0 carries S_W1 * S_PRE[0] = 16, others carry S_PRE[l]
S_POST = (1.0 / 16.0, 1.0 / 64.0, 1.0 / 128.0)

_PROGRAM = None
_LAST_RES = None


def _build_program():
    nc = bacc.Bacc(None)
    f32, fp8 = mybir.dt.float32, mybir.dt.float8e4

    xT_d = nc.dram_tensor("xT", [KC2, 128, 2, ND], fp8, kind="ExternalInput")
    W1c_d = nc.dram_tensor("W1c", [128, KC2 * 2 * HP], fp8, kind="ExternalInput")
    A_d = nc.dram_tensor("A", [KC2, 128, 2, ND], fp8, kind="ExternalInput")
    disr_d = nc.dram_tensor("disr", [H, ND], f32, kind="ExternalInput")
    W2_d = nc.dram_tensor("W2", [H, H], f32, kind="ExternalInput")
    W3_d = nc.dram_tensor("W3", [H, C], f32, kind="ExternalInput")
    b1_d = nc.dram_tensor("b1", [H, 1], f32, kind="ExternalInput")
    b2_d = nc.dram_tensor("b2", [H, 1], f32, kind="ExternalInput")
    b3_d = nc.dram_tensor("b3", [C, 1], f32, kind="ExternalInput")
    I_d = nc.dram_tensor("ident", [128, 128], f32, kind="ExternalInput")
    out_d = nc.dram_tensor("out", [ND, C], f32, kind="ExternalOutput")

    cc_in = [nc.dram_tensor(f"cc{l}_in", [128, RB * HP], fp8)
             for l in range(3)]
    cc_out = [nc.dram_tensor(f"cc{l}_out", [NCORES * 128, RB * HP], fp8,
                             addr_space="Shared")
              for l in range(3)]
    groups = [list(range(NCORES))]

    with tile.TileContext(nc) as tc:
        with (
            tc.tile_pool(name="const", bufs=1) as constp,
            tc.tile_pool(name="acache", bufs=1) as acp,
            tc.tile_pool(name="mv", bufs=4) as mvp,
            tc.tile_pool(name="gt", bufs=1) as gtp,
            tc.tile_pool(name="work", bufs=1) as work,
            tc.tile_pool(name="psb", bufs=1, space="PSUM") as psb,
            tc.tile_pool(name="pst", bufs=2, space="PSUM") as pst,
        ):
            W1c = constp.tile([128, KC2, 2, HP], fp8)
            disr = constp.tile([H, ND], f32)
            W2t = constp.tile([H, H], f32)
            W3t = constp.tile([H, C], f32)
            b1t = constp.tile([H, 1], f32)
            b2t = constp.tile([H, 1], f32)
            b3t = constp.tile([C, 1], f32)
            ident = constp.tile([128, 128], f32)
            nc.sync.dma_start(W1c[:], W1c_d[:].rearrange("p (c t f) -> p c t f",
                                                         t=2, f=HP))
            nc.sync.dma_start(disr[:], disr_d[:])
            nc.sync.dma_start(W2t[:], W2_d[:])
            nc.sync.dma_start(W3t[:], W3_d[:])
            nc.sync.dma_start(b1t[:], b1_d[:])
            nc.sync.dma_start(b2t[:], b2_d[:])
            nc.sync.dma_start(b3t[:], b3_d[:])
            nc.sync.dma_start(ident[:], I_d[:])

            acache = acp.tile([128, NCACHE, 2, ND], fp8)
            g_t = gtp.tile([128, KC2, 2, HP], fp8, tag="g")

            def g_lhsT(cc):
                return g_t[:, cc, :, :]

            gown = work.tile([128, RB, HP], fp8, tag="gown")
            nc.vector.memset(gown[:, :, H:HP], 0.0)

            # ---- GEMM1: h1T[36, 2048] += W1[cc]^T @ xT[cc]  (DoubleRow) ----
            hT = psb.tile([HP, ND], f32, tag="big")
            for cp in range(KC2 // 2):
                xt = mvp.tile([128, 2, 2, ND], fp8, tag="mv")
                nc.sync.dma_start(
                    xt[:],
                    xT_d[2 * cp:2 * cp + 2, :, :, :].rearrange(
                        "c p t d -> p c t d"),
                )
                for i in range(2):
                    cc = 2 * cp + i
                    for q in range(4):
                        nc.tensor.matmul(
                            hT[0:HP, q * 512:(q + 1) * 512],
                            W1c[:, cc, :, :],
                            xt[:, i, :, q * 512:(q + 1) * 512],
                            start=(cc == 0),
                            stop=(cc == KC2 - 1),
                            perf_mode=DR,
                        )

            # prefetch the A cache fills now: they flow on SP during the
            # layer-0 gather window while PE is idle on the collective
            for cf in range(NCACHE // 2):
                nc.sync.dma_start(
                    acache[:, 2 * cf:2 * cf + 2, :, :],
                    A_d[2 * cf:2 * cf + 2, :, :, :].rearrange(
                        "c p t d -> p c t d"),
                )

            for layer in range(3):
                F = H if layer < 2 else C
                # ---- prescale by dis (own rows), quantize, share ----
                gT = work.tile([H, ND], f32, tag="w1")
                nc.vector.tensor_tensor(
                    gT[0:F, :], hT[0:F, :], disr[0:F, :], mybir.AluOpType.mult
                )
                if layer == 2:
                    nc.vector.memset(gown[:, :, C:H], 0.0)
                for rb in range(RB):
                    sl = slice(rb * 128, (rb + 1) * 128)
                    tp = pst.tile([128, H], f32, tag="tp")
                    nc.tensor.transpose(
                        tp[:, 0:F],
                        gT[0:F, sl],
                        ident[0:F, 0:F],
                    )
                    nc.vector.tensor_scalar(
                        gown[:, rb, 0:F], tp[:, 0:F],
                        float(S_PRE[layer]), None, mybir.AluOpType.mult,
                    )
                nc.scalar.dma_start(
                    cc_in[layer][:].rearrange("p (r f) -> p r f", f=HP), gown[:]
                )
                nc.gpsimd.collective_compute(
                    "AllGather",
                    mybir.AluOpType.bypass,
                    replica_groups=groups,
                    ins=[cc_in[layer][:]],
                    outs=[cc_out[layer][:]],
                )
                nc.scalar.dma_start(
                    g_t[:].rearrange("p (c lt) t f -> p c (lt t f)", c=8),
                    cc_out[layer][:].rearrange("(c p) ltf -> p c ltf", p=128),
                )


                # ---- dense aggregation: aggT[F, 2048] += g[cc]^T @ A[cc] ----
                # streamed chunks first (fresh DMA), cached chunks last: frees
                # stream buffers early so the next layer's prefetch can run
                # during this layer's cached phase and the next collective.
                aggT = psb.tile([HP, ND], f32, tag="big")
                FS = HP
                nmm = 0

                def agg_mm(cc, a_ap):
                    nonlocal nmm
                    for q in range(4):
                        nc.tensor.matmul(
                            aggT[0:FS, q * 512:(q + 1) * 512],
                            g_lhsT(cc),
                            a_ap[:, :, q * 512:(q + 1) * 512],
                            start=(nmm == 0),
                            stop=(nmm == KC2 - 1),
                            perf_mode=DR,
                        )
                    nmm += 1

                def stream_pair(sp):
                    at = mvp.tile([128, 2, 2, ND], fp8, tag="mv")
                    c0 = NCACHE + 2 * sp
                    nc.sync.dma_start(
                        at[:],
                        A_d[c0:c0 + 2, :, :, :].rearrange("c p t d -> p c t d"),
                    )
                    for i in range(2):
                        agg_mm(c0 + i, at[:, i, :, :])

                # interleave ~2 cached chunks per streamed pair so the PE
                # never outruns the stream DMA; prime with cached chunks
                npairs = (KC2 - NCACHE) // 2
                k = 0
                for sp in range(npairs):
                    for _ in range(2):
                        if k < NCACHE:
                            agg_mm(k, acache[:, k, :, :])
                            k += 1
                    stream_pair(sp)
                while k < NCACHE:
                    agg_mm(k, acache[:, k, :, :])
                    k += 1

                if layer < 2:
                    # in_{l+1}T = relu(s_post * (dis*aggT) + b)
                    tmp = work.tile([H, ND], f32, tag="w2")
                    nc.vector.tensor_tensor(
                        tmp[:], aggT[0:H, :], disr[:], mybir.AluOpType.mult
                    )
                    inT = work.tile([H, ND], f32, tag="w1")
                    nc.scalar.activation(
                        inT[:], tmp[:], mybir.ActivationFunctionType.Relu,
                        bias=b1t[:] if layer == 0 else b2t[:],
                        scale=float(S_POST[layer]),
                    )
                    # next-layer GEMM: hT = W^T @ inT  (K = 36, fp32)
                    Wt = W2t if layer == 0 else W3t
                    Fn = H if layer == 0 else C
                    hT = psb.tile([HP, ND], f32, tag="big")
                    for q in range(4):
                        nc.tensor.matmul(
                            hT[0:Fn, q * 512:(q + 1) * 512],
                            Wt[:],
                            inT[:, q * 512:(q + 1) * 512],
                            start=True,
                            stop=True,
                        )
                else:
                    # logitsT = s_post * (dis*aggT) + b3 ; softmax over classes
                    tmp = work.tile([H, ND], f32, tag="w2")
                    nc.vector.tensor_tensor(
                        tmp[0:C, :], aggT[0:C, :], disr[0:C, :],
                        mybir.AluOpType.mult,
                    )
                    logT = work.tile([C, ND], f32, tag="w1")
                    nc.vector.tensor_scalar(
                        logT[:], tmp[0:C, :],
                        float(S_POST[layer]), b3t[:],
                        mybir.AluOpType.mult, mybir.AluOpType.add,
                    )
                    # transpose to natural [2048, 16]
                    onat = work.tile([128, RB, C], f32, tag="onat")
                    for rb in range(RB):
                        tp = pst.tile([128, H], f32, tag="tp")
                        nc.tensor.transpose(
                            tp[:, 0:C],
                            logT[:, rb * 128:(rb + 1) * 128],
                            ident[0:C, 0:C],
                        )
                        nc.vector.tensor_copy(onat[:, rb, :], tp[:, 0:C])
                    # softmax along the class (free) dim
                    negmax = work.tile([128, RB], f32, tag="negmax")
                    nc.vector.tensor_reduce(
                        negmax[:], onat[:], axis=mybir.AxisListType.X,
                        op=mybir.AluOpType.max, negate=True,
                    )
                    expv = work.tile([128, RB, C], f32, tag="expv")
                    ssum = work.tile([128, RB], f32, tag="ssum")
                    for rb in range(RB):
                        nc.scalar.activation(
                            expv[:, rb, :], onat[:, rb, :],
                            mybir.ActivationFunctionType.Exp,
                            bias=negmax[:, rb:rb + 1],
                            accum_out=ssum[:, rb:rb + 1],
                        )
                    rsum = work.tile([128, RB], f32, tag="rsum")
                    nc.vector.reciprocal(rsum[:], ssum[:])
                    prob = work.tile([128, RB, C], f32, tag="onat")
                    for rb in range(RB):
                        nc.vector.tensor_scalar(
                            prob[:, rb, :], expv[:, rb, :],
                            rsum[:, rb:rb + 1], None, mybir.AluOpType.mult,
                        )
                    nc.scalar.dma_start(
                        out_d[:].rearrange("(b p) f -> p b f", p=128), prob[:]
                    )

    nc.finalize()
    return nc


def _get_program():
    global _PROGRAM
    if _PROGRAM is None:
        _PROGRAM = _build_program()
    return _PROGRAM


def kernel(x, edge_index, W1, b1, W2, b2, W3, b3, _profile=False):
    x = np.asarray(x, dtype=np.float32)
    edge_index = np.asarray(edge_index)
    W1 = np.asarray(W1, dtype=np.float32)
    W2 = np.asarray(W2, dtype=np.float32)
    W3 = np.asarray(W3, dtype=np.float32)
    b1 = np.asarray(b1, dtype=np.float32)
    b2 = np.asarray(b2, dtype=np.float32)
    b3 = np.asarray(b3, dtype=np.float32)

    # ---- graph preprocessing (host) ----
    loop = np.arange(N, dtype=np.int64)
    src = np.concatenate([edge_index[0].astype(np.int64), loop])
    dst = np.concatenate([edge_index[1].astype(np.int64), loop])
    deg = np.bincount(dst, minlength=N).astype(np.float32)
    dis = (1.0 / np.sqrt(np.maximum(deg, np.float32(1.0)))).astype(np.float32)

    # W1 chunked for DoubleRow: [p, cc, t, f] with src row = 256*cc + 128*t + p
    W1p = np.zeros((N, HP), np.float32)
    W1p[:, 0:H] = W1 * np.float32(S_W1)
    W1c = np.ascontiguousarray(
        W1p.astype(FP8)
        .reshape(KC2, 2, 128, HP).transpose(2, 0, 1, 3)
    ).reshape(128, KC2 * 2 * HP)
    ident = np.eye(128, dtype=np.float32)

    core_dst = dst // ND
    in_maps = []
    for c in range(NCORES):
        m = core_dst == c
        # A[src, local_dst] edge multiplicities, chunked [cc, p, t, d]
        lin = src[m] * ND + (dst[m] - c * ND)
        cnt = np.bincount(lin, minlength=N * ND).astype(np.float32)
        A8 = np.ascontiguousarray(
            cnt.astype(FP8).reshape(KC2, 2, 128, ND).transpose(0, 2, 1, 3)
        )
        xT = np.ascontiguousarray(
            x[c * ND:(c + 1) * ND, :].astype(FP8).T
            .reshape(KC2, 2, 128, ND).transpose(0, 2, 1, 3)
        )
        disr = np.ascontiguousarray(
            np.broadcast_to(dis[c * ND:(c + 1) * ND][None, :], (H, ND))
        )
        in_maps.append({
            "xT": xT,
            "W1c": W1c,
            "A": A8,
            "disr": disr,
            "W2": W2,
            "W3": W3,
            "b1": b1.reshape(H, 1),
            "b2": b2.reshape(H, 1),
            "b3": b3.reshape(C, 1),
            "ident": ident,
        })

    nc = _get_program()
    global _LAST_RES
    res = run_bass_kernel_spmd(nc, in_maps, list(range(NCORES)),
                               trace=bool(_profile))
    _LAST_RES = res
    out = np.concatenate([res.results[c]["out"] for c in range(NCORES)], axis=0)
    if _profile:
        return out, res.exec_time_ns
    return out
